# revision 1
# baseline (speedup 1.0000x reference)
"""Trainium2 Bass kernel: Conformer MHSA with relative positional encoding.

Shapes (hardcoded): B=8, T=1024, E=512, H=8, DH=64.
Sharding: data-parallel over batch -- one batch element per NeuronCore (8 cores).

Math notes (all validated against the reference in numpy):
  - LayerNorm gamma/beta are folded into the projection weights/biases on host.
  - The relative-position term bd[i,j] = q_v[i] . pe(i-j) is decomposed via
    angle-addition:  pe(i-j) sin/cos terms split into products of f(i) and g(j),
    so bd = [A|B][i] . [cos_j|sin_j][j] with A,B elementwise-built from q_v.
    This folds bd INTO the main logits matmul (contraction 64 -> 128) and makes
    the rel-shift implicit.  No (T, 2T-1) matmul, no shift pass.
  - k-projection bias is dropped entirely (adds a per-row constant to logits,
    softmax-invariant).  v bias is folded into the output-projection bias.
  - Softmax skips max-subtraction (logits are bounded ~|10|; exp is safe in f32)
    and the denominator rides the AV matmul as a ones-column (M=65); the
    division is applied to the small (64,T) per-head output, not the (T,T)
    weights.
  - sequence_mask folded into v/ones columns (zeroed rows drop out of both the
    numerator and denominator), which is exactly masked softmax.
"""

import os
import sys

import numpy as np

sys.path.insert(0, "/opt/trn_rl_repo")

T = 1024
E = 512
H = 8
DH = 64
F = DH // 2  # 32
NT = T // 128  # 8 i/j tiles
NC = E // 128  # 4 c/e tiles
LN_EPS = 1e-5
N_CORES = 8

_CACHE = {}


def _build_nc():
    import concourse.bass as bass
    import concourse.tile as tile
    from concourse import bacc, library_config, mybir

    f32 = mybir.dt.float32
    f32r = mybir.dt.float32r
    Alu = mybir.AluOpType
    Act = mybir.ActivationFunctionType

    def r(ap):
        return ap.bitcast(f32r)

    nc = bacc.Bacc("TRN2", target_bir_lowering=False, debug=False, num_devices=N_CORES)

    x_d = nc.declare_dram_parameter("x", [T, E], f32, isOutput=False)
    qwT_d = nc.declare_dram_parameter("qwT", [E, E], f32r, isOutput=False)
    kwT_d = nc.declare_dram_parameter("kwT", [E, E], f32r, isOutput=False)
    vwT_d = nc.declare_dram_parameter("vwT", [E, E], f32r, isOutput=False)
    owT_d = nc.declare_dram_parameter("owT", [E, E], f32r, isOutput=False)
    pb_d = nc.declare_dram_parameter("pbias", [128, 8], f32, isOutput=False)
    tbl_d = nc.declare_dram_parameter("tblcs", [DH, T], f32r, isOutput=False)
    tblS_d = nc.declare_dram_parameter("tblsin", [128, T], f32, isOutput=False)
    tblC_d = nc.declare_dram_parameter("tblcos", [128, T], f32, isOutput=False)
    ob_d = nc.declare_dram_parameter("obias", [128, E], f32, isOutput=False)
    mask_d = nc.declare_dram_parameter("maskt", [128, NT], f32, isOutput=False)
    vones_d = nc.declare_dram_parameter("vones", [128, NT * 512], f32r, isOutput=False)
    id_d = nc.declare_dram_parameter("ident", [128, 128], f32r, isOutput=False)
    out_d = nc.declare_dram_parameter("out", [T, E], f32, isOutput=True)

    x_t = x_d[:].rearrange("(t p) e -> t p e", p=128)
    out_t = out_d[:].rearrange("(t p) e -> t p e", p=128)

    with tile.TileContext(nc) as tc:
        from contextlib import ExitStack

        with ExitStack() as ctx:
            consts = ctx.enter_context(tc.tile_pool(name="consts", bufs=1))
            sb = ctx.enter_context(tc.tile_pool(name="sb", bufs=1))
            ps = ctx.enter_context(tc.tile_pool(name="ps", bufs=1, space="PSUM"))

            # ---- constants ----
            wq = [consts.tile([128, E], f32r, tag=f"wq{c}", name=f"wq{c}") for c in range(NC)]
            wk = [consts.tile([128, E], f32r, tag=f"wk{c}", name=f"wk{c}") for c in range(NC)]
            wv = [consts.tile([128, E], f32r, tag=f"wv{c}", name=f"wv{c}") for c in range(NC)]
            wo = [consts.tile([128, E], f32r, tag=f"wo{c}", name=f"wo{c}") for c in range(NC)]
            for c in range(NC):
                nc.sync.dma_start(wq[c][:], qwT_d[:].rearrange("(t p) e -> t p e", p=128)[c])
                nc.sync.dma_start(wk[c][:], kwT_d[:].rearrange("(t p) e -> t p e", p=128)[c])
                nc.sync.dma_start(wv[c][:], vwT_d[:].rearrange("(t p) e -> t p e", p=128)[c])
                nc.sync.dma_start(wo[c][:], owT_d[:].rearrange("(t p) e -> t p e", p=128)[c])
            # sin/cos tables replicated on all four 32-partition groups so DVE
            # ops can read them at whatever base partition the operand lives.
            tblS = consts.tile([128, T], f32, tag="tblS")
            nc.sync.dma_start(tblS[:], tblS_d[:])
            tblC = consts.tile([128, T], f32, tag="tblC")
            nc.sync.dma_start(tblC[:], tblC_d[:])
            pb = consts.tile([128, 8], f32, tag="pb")
            nc.sync.dma_start(pb[:], pb_d[:])
            ob = consts.tile([128, E], f32, tag="ob")
            nc.sync.dma_start(ob[:], ob_d[:])
            mk = consts.tile([128, NT], f32, tag="mk")
            nc.sync.dma_start(mk[:], mask_d[:])
            ident = consts.tile([128, 128], f32r, tag="ident")
            nc.sync.dma_start(ident[:], id_d[:])
            epsc = consts.tile([128, 1], f32, tag="epsc")
            nc.vector.memset(epsc[:], LN_EPS)

            # ---- phase B: LayerNorm (z = (x-mu)*rstd; gamma/beta folded on host)
            z_tiles = []
            for t in range(NT):
                xt = sb.tile([128, E], f32, tag="x", bufs=2)
                nc.sync.dma_start(xt[:], x_t[t])
                st = sb.tile([128, 6], f32, tag="st", bufs=2)
                nc.vector.bn_stats(st[:], xt[:])
                mv = sb.tile([128, 2], f32, tag="mv", bufs=2)
                nc.vector.bn_aggr(mv[:], st[:])
                sd = sb.tile([128, 1], f32, tag="sd", bufs=2)
                nc.scalar.activation(sd[:], mv[:, 1:2], Act.Sqrt, bias=epsc[:], scale=1.0)
                rstd = sb.tile([128, 1], f32, tag="rstd", bufs=4)
                nc.vector.reciprocal(rstd[:], sd[:])
                nmr = sb.tile([128, 1], f32, tag="nmr", bufs=4)
                # nmr = (mu * -1) * rstd
                nc.vector.scalar_tensor_tensor(
                    nmr[:], mv[:, 0:1], -1.0, rstd[:], Alu.mult, Alu.mult
                )
                zt = sb.tile([128, E], f32r, tag="z", bufs=2)
                nc.scalar.activation(zt[:], xt[:], Act.Identity, bias=nmr[:], scale=rstd[:])
                z_tiles.append(zt)

            # ---- phase C: transpose z -> zT (c on partitions, i free)
            zT = [sb.tile([128, T], f32r, tag=f"zT{c}", name=f"zT{c}") for c in range(NC)]
            for t in range(NT):
                for c in range(NC):
                    pt = ps.tile([128, T], f32, tag="pA", bufs=3)
                    nc.tensor.transpose(
                        r(pt[:, 0:128]),
                        r(z_tiles[t][:, c * 128 : (c + 1) * 128]),
                        r(ident[:]),
                    )
                    nc.vector.tensor_copy(zT[c][:, t * 128 : (t + 1) * 128], pt[:, 0:128])

            # ---- phase D: projections ----
            # Q and K: transposed outputs (e on partitions), per head-pair e-tile.
            qbig = {}
            kbig = {}

            def emit_qk(et):
                # Q
                psq = ps.tile([128, T], f32, tag="pA", bufs=3, name=f"psq{et}")
                for ic in range(2):
                    for c in range(NC):
                        nc.tensor.matmul(
                            psq[:, ic * 512 : (ic + 1) * 512],
                            r(wq[c][:, et * 128 : (et + 1) * 128]),
                            r(zT[c][:, ic * 512 : (ic + 1) * 512]),
                            start=(c == 0),
                            stop=(c == NC - 1),
                        )
                for half in range(2):
                    h = 2 * et + half
                    qb = sb.tile([128, T], f32r, tag="qbig", bufs=4, name=f"qbig{h}")
                    qbig[h] = qb
                    p0 = half * 64  # base partition of this head's slice in psum
                    # rows 0:64 <- q_u = psum + bias_qu   (pb col et)
                    if half == 0:
                        nc.scalar.activation(
                            qb[0:64, :],
                            psq[0:64, :],
                            Act.Identity,
                            bias=pb[0:64, et : et + 1],
                            scale=1.0,
                        )
                    else:
                        # evac at source partitions, then DMA-shift 64 -> 0
                        qtmp = sb.tile([128, T], f32r, tag="qtmp", bufs=1)
                        nc.scalar.activation(
                            qtmp[64:128, :],
                            psq[64:128, :],
                            Act.Identity,
                            bias=pb[64:128, et : et + 1],
                            scale=1.0,
                        )
                        nc.sync.dma_start(qb[0:64, :], qtmp[64:128, :])
                    # A = (qs+bv_s)*sin + (qc+bv_c)*cos -> rows 64:96
                    # B = (qc+bv_c)*sin - (qs+bv_s)*cos -> rows 96:128
                    # Products are built at the source partitions, DMA-shifted
                    # to the target partition group, then combined there.
                    sr = slice(p0, p0 + 32)  # qs partitions
                    cr = slice(p0 + 32, p0 + 64)  # qc partitions
                    qs = psq[sr, :]
                    qc = psq[cr, :]
                    bqs = pb[sr, 4 + et : 5 + et]
                    bqc = pb[cr, 4 + et : 5 + et]
                    absrc = sb.tile([128, 2 * T], f32, tag="absrc", bufs=1)
                    u = absrc[sr, 0:T]
                    y = absrc[sr, T : 2 * T]
                    x = absrc[cr, 0:T]
                    w = absrc[cr, T : 2 * T]
                    nc.vector.scalar_tensor_tensor(u, qs, bqs, tblS[sr, :], Alu.add, Alu.mult)
                    nc.vector.scalar_tensor_tensor(y, qs, bqs, tblC[sr, :], Alu.add, Alu.mult)
                    nc.vector.scalar_tensor_tensor(x, qc, bqc, tblS[cr, :], Alu.add, Alu.mult)
                    nc.vector.scalar_tensor_tensor(w, qc, bqc, tblC[cr, :], Alu.add, Alu.mult)
                    abd = sb.tile([128, 2 * T], f32, tag="abd", bufs=1)
                    nc.sync.dma_start(abd[64:96, 0:T], u)
                    nc.sync.dma_start(abd[64:96, T : 2 * T], w)
                    nc.sync.dma_start(abd[96:128, 0:T], x)
                    nc.sync.dma_start(abd[96:128, T : 2 * T], y)
                    nc.vector.tensor_add(qb[64:96, :], abd[64:96, 0:T], abd[64:96, T : 2 * T])
                    nc.vector.tensor_sub(qb[96:128, :], abd[96:128, 0:T], abd[96:128, T : 2 * T])
                # K
                psk = ps.tile([128, T], f32, tag="pA", bufs=3, name=f"psk{et}")
                for ic in range(2):
                    for c in range(NC):
                        nc.tensor.matmul(
                            psk[:, ic * 512 : (ic + 1) * 512],
                            r(wk[c][:, et * 128 : (et + 1) * 128]),
                            r(zT[c][:, ic * 512 : (ic + 1) * 512]),
                            start=(c == 0),
                            stop=(c == NC - 1),
                        )
                for half in range(2):
                    h = 2 * et + half
                    kb = sb.tile([128, T], f32r, tag="kbig", bufs=4, name=f"kbig{h}")
                    kbig[h] = kb
                    # rows 0:64 <- k (no bias; softmax-invariant)
                    if half == 0:
                        nc.scalar.copy(kb[0:64, :], psk[0:64, :])
                    else:
                        ktmp = sb.tile([128, T], f32r, tag="qtmp", bufs=1)
                        nc.scalar.copy(ktmp[64:128, :], psk[64:128, :])
                        nc.sync.dma_start(kb[0:64, :], ktmp[64:128, :])
                    # rows 64:128 <- [cosT; sinT] position tables
                    nc.sync.dma_start(kb[64:128, :], tbl_d[:])

            # V: natural layout (j on partitions), with ones column per head.
            v_ext = []

            def emit_v(jt):
                psv = ps.tile([128, T], f32, tag="pA", bufs=3, name=f"psv{jt}")
                for c in range(NC):
                    nc.tensor.matmul(
                        psv[:, 0:512],
                        r(zT[c][:, jt * 128 : (jt + 1) * 128]),
                        r(wv[c][:]),
                        start=(c == 0),
                        stop=(c == NC - 1),
                    )
                vx = sb.tile([128, H * 128], f32r, tag=f"vx{jt}", name=f"vx{jt}")
                v_ext.append(vx)
                vx3 = vx[:].rearrange("p (h f) -> p h f", f=128)
                # ones columns, pre-masked on host (one strided DMA)
                nc.sync.dma_start(
                    vx3[:, :, DH:128], vones_d[:, jt * 512 : (jt + 1) * 512]
                )
                # v columns, masked (scale is per-partition = per-j)
                nc.scalar.activation(
                    vx3[:, :, 0:DH],
                    psv[:, 0:512].rearrange("p (h f) -> p h f", f=DH),
                    Act.Copy,
                    scale=mk[:, jt : jt + 1],
                )

            # ---- phase E: per-head attention ----
            oT = []

            def emit_head(h):
                p_tiles = []
                for jt in range(NT):
                    psl = ps.tile([128, T], f32, tag="pA", bufs=3, name=f"psl{h}_{jt}")
                    for ic in range(2):
                        nc.tensor.matmul(
                            psl[:, ic * 512 : (ic + 1) * 512],
                            r(kbig[h][:, jt * 128 : (jt + 1) * 128]),
                            r(qbig[h][:, ic * 512 : (ic + 1) * 512]),
                            start=True,
                            stop=True,
                        )
                    pexp = sb.tile([128, T], f32r, tag="P", bufs=5)
                    p_tiles.append(pexp)
                    nc.scalar.activation(pexp[:], psl[:], Act.Exp, scale=0.125)
                psav = ps.tile([128, T], f32, tag="pB", bufs=1)
                for jt in range(NT):
                    for ic in range(2):
                        nc.tensor.matmul(
                            psav[:, ic * 512 : (ic + 1) * 512],
                            r(v_ext[jt][:, h * 128 : (h + 1) * 128]),
                            r(p_tiles[jt][:, ic * 512 : (ic + 1) * 512]),
                            start=(jt == 0),
                            stop=(jt == NT - 1),
                        )
                # psav rows 64:128 hold the softmax denominator replicated on
                # 64 partitions (ones-block columns of v_ext).  Reciprocal at
                # the matching partitions, DMA-shift down to 0:64, multiply.
                rr = sb.tile([128, T], f32, tag="rr", bufs=2)
                nc.vector.reciprocal(rr[DH:128, :], psav[DH:128, :])
                nc.sync.dma_start(rr[0:DH, :], rr[DH:128, :])
                if h % 2 == 0:
                    ot = sb.tile([128, T], f32r, tag=f"oT{h // 2}", name=f"oT{h // 2}")
                    oT.append(ot)
                    nc.vector.tensor_mul(ot[0:DH, :], psav[0:DH, :], rr[0:DH, :])
                else:
                    otmp = sb.tile([DH, T], f32r, tag="otmp", bufs=1)
                    nc.vector.tensor_mul(otmp[:], psav[0:DH, :], rr[0:DH, :])
                    nc.sync.dma_start(oT[h // 2][DH : 2 * DH, :], otmp[:])

            # Emission order interleaves QK projection and attention so that
            # qbig/kbig slot recycling (bufs=4) never blocks an in-order engine
            # behind work that depends on a later release (deadlock otherwise).
            emit_qk(0)
            emit_qk(1)
            for jt in range(NT):
                emit_v(jt)
            for h in range(4):
                emit_head(h)
            emit_qk(2)
            emit_qk(3)
            for h in range(4, H):
                emit_head(h)

            # ---- phase F: output projection ----
            for it in range(NT):
                psy = ps.tile([128, T], f32, tag="pA", bufs=3)
                for ft in range(NC):
                    nc.tensor.matmul(
                        psy[:, 0:512],
                        r(oT[ft][:, it * 128 : (it + 1) * 128]),
                        r(wo[ft][:]),
                        start=(ft == 0),
                        stop=(ft == NC - 1),
                    )
                yt = sb.tile([128, E], f32, tag="y", bufs=3)
                nc.vector.tensor_add(yt[:], psy[:, 0:512], ob[:])
                nc.sync.dma_start(out_t[it], yt[:])

    if not nc.is_finalized():
        nc.finalize()
    return nc


def _host_prep(inputs):
    """Fold LN gamma/beta + biases into weights; build tables. Returns in_maps."""
    x = np.asarray(inputs["input_tensor"], np.float32)  # (B, T, E)
    mask = np.asarray(inputs["sequence_mask"])  # (B, T) bool
    gamma = np.asarray(inputs["ln_scale"], np.float32)
    beta = np.asarray(inputs["ln_bias"], np.float32)
    q_w = np.asarray(inputs["q_w"], np.float32)
    k_w = np.asarray(inputs["k_w"], np.float32)
    v_w = np.asarray(inputs["v_w"], np.float32)
    in_b = np.asarray(inputs["in_proj_bias"], np.float32)
    out_w = np.asarray(inputs["out_w"], np.float32)
    out_b = np.asarray(inputs["out_b"], np.float32)
    pos_u = np.asarray(inputs["pos_bias_u"], np.float32).reshape(-1)
    pos_v = np.asarray(inputs["pos_bias_v"], np.float32).reshape(-1)

    bias_k, bias_q, bias_v = np.split(in_b, 3)  # torch unpack order

    qw_eff = q_w * gamma[None, :]
    kw_eff = k_w * gamma[None, :]
    vw_eff = v_w * gamma[None, :]
    bias_qu = q_w @ beta + bias_q + pos_u
    bias_qv = q_w @ beta + bias_q + pos_v
    bias_v_full = v_w @ beta + bias_v
    out_b_eff = out_b + out_w @ bias_v_full

    qwT = np.ascontiguousarray(qw_eff.T)
    kwT = np.ascontiguousarray(kw_eff.T)
    vwT = np.ascontiguousarray(vw_eff.T)
    owT = np.ascontiguousarray(out_w.T)

    pbias = np.zeros((128, 8), np.float32)
    for et in range(NC):
        pbias[:, et] = bias_qu[et * 128 : (et + 1) * 128]
        pbias[:, 4 + et] = bias_qv[et * 128 : (et + 1) * 128]

    inv_freq = (1.0 / (10000.0 ** (np.arange(0, DH, 2, dtype=np.float64) / DH))).astype(
        np.float64
    )
    ang = np.outer(inv_freq, np.arange(T, dtype=np.float64))  # (32, T)
    cosT = np.cos(ang)
    sinT = np.sin(ang)
    tblcs = np.concatenate([cosT, sinT], axis=0).astype(np.float32)  # (64, T)
    tblsin = np.tile(sinT, (4, 1)).astype(np.float32)  # (128, T)
    tblcos = np.tile(cosT, (4, 1)).astype(np.float32)  # (128, T)

    obias = np.tile(out_b_eff.reshape(1, E), (128, 1)).astype(np.float32)
    ident = np.eye(128, dtype=np.float32)

    shared = {
        "qwT": qwT,
        "kwT": kwT,
        "vwT": vwT,
        "owT": owT,
        "pbias": pbias,
        "tblcs": tblcs,
        "tblsin": tblsin,
        "tblcos": tblcos,
        "obias": obias,
        "ident": ident,
    }
    in_maps = []
    for b in range(N_CORES):
        mt = np.zeros((128, NT), np.float32)
        mb = mask[b].astype(np.float32)
        for jt in range(NT):
            mt[:, jt] = mb[jt * 128 : (jt + 1) * 128]
        vones = np.repeat(mt.T.reshape(NT, 128, 1), 512, axis=2)  # (NT,128,512)
        vones = np.ascontiguousarray(vones.transpose(1, 0, 2).reshape(128, NT * 512))
        in_maps.append(
            {"x": np.ascontiguousarray(x[b]), "maskt": mt, "vones": vones, **shared}
        )
    return in_maps


def kernel(**inputs) -> np.ndarray:
    from concourse.bass_utils import run_bass_kernel_spmd

    in_maps = _host_prep(inputs)
    if "nc" not in _CACHE:
        _CACHE["nc"] = _build_nc()
    trace = os.environ.get("KERNEL_TRACE", "0") == "1"
    res = run_bass_kernel_spmd(
        _CACHE["nc"], in_maps, core_ids=list(range(N_CORES)), trace=trace
    )
    _CACHE["last_result"] = res
    out = np.stack([res.results[i]["out"] for i in range(N_CORES)], axis=0)
    return out.astype(np.float32)



# revision 8
# speedup vs baseline: 1.5062x; 1.5062x over previous
"""Trainium2 Bass kernel: Conformer MHSA with relative positional encoding.

Shapes (hardcoded): B=8, T=1024, E=512, H=8, DH=64.
Sharding: data-parallel over batch -- one batch element per NeuronCore (8 cores).

v2 (bf16 + restructure). Key points vs v1:
  - All matmuls in bf16 (enables FWL fast weight load; halves SBUF traffic;
    DVE gets 2x on 16-bit ops). Accuracy budget is rel<2e-2; measured ~2e-3.
  - LayerNorm gamma/beta folded into projection weights/biases on host.
  - Rel-pos term folded into the logits matmul via angle-addition
    (contraction 64 -> 128), making the rel-shift implicit.
  - k bias dropped (softmax-invariant); v bias folded into output bias.
  - Softmax denominator rides the AV matmul as masked ones-columns.
    v_ext column order is [v|ones] for even heads and [ones|v] for odd heads,
    so the odd head's numerator lands on PSUM partitions 64:128 -- exactly the
    rows the paired oT tile needs.  No partition-shift DMAs anywhere:
    cross-partition moves use engine copies / mixed-space (SBUF+PSUM)
    tensor_tensor operands, which hardware allows at different bases.
  - reciprocal_approx_fast (18-bit) for the softmax denominator.
  - exp on the scalar engine; everything else off it so exp streams.
"""

import os
import sys

import numpy as np

sys.path.insert(0, "/opt/trn_rl_repo")

T = 1024
E = 512
H = 8
DH = 64
NT = T // 128  # 8 i/j tiles
NC = E // 128  # 4 c/e tiles
LN_EPS = 1e-5
N_CORES = 8

_CACHE = {}


def _build_nc():
    import concourse.bass as bass
    import concourse.tile as tile
    from concourse import bacc, mybir

    f32 = mybir.dt.float32
    f32r = mybir.dt.float32r
    bf16 = mybir.dt.bfloat16
    Alu = mybir.AluOpType
    Act = mybir.ActivationFunctionType

    def r(ap):
        return ap.bitcast(f32r)

    nc = bacc.Bacc("TRN2", target_bir_lowering=False, debug=False, num_devices=N_CORES)

    x_d = nc.declare_dram_parameter("x", [T, E], f32, isOutput=False)
    qwT_d = nc.declare_dram_parameter("qwT", [E, E], bf16, isOutput=False)
    kwT_d = nc.declare_dram_parameter("kwT", [E, E], bf16, isOutput=False)
    vwT_d = nc.declare_dram_parameter("vwT", [E, E], bf16, isOutput=False)
    owT_d = nc.declare_dram_parameter("owT", [E, E], bf16, isOutput=False)
    pb_d = nc.declare_dram_parameter("pbias", [128, 8], f32, isOutput=False)
    tblS_d = nc.declare_dram_parameter("tblsin", [128, T], bf16, isOutput=False)
    tblC_d = nc.declare_dram_parameter("tblcos", [128, T], bf16, isOutput=False)
    tblK_d = nc.declare_dram_parameter("tblk", [DH, T], bf16, isOutput=False)
    ob_d = nc.declare_dram_parameter("obias", [128, E], f32, isOutput=False)
    mask_d = nc.declare_dram_parameter("maskt", [128, NT], f32, isOutput=False)
    vones_d = nc.declare_dram_parameter("vones", [128, NT * 512], bf16, isOutput=False)
    id_d = nc.declare_dram_parameter("ident", [128, 128], f32r, isOutput=False)
    out_d = nc.declare_dram_parameter("out", [T, E], f32, isOutput=True)

    x_t = x_d[:].rearrange("(t p) e -> t p e", p=128)
    out_t = out_d[:].rearrange("(t p) e -> t p e", p=128)

    with tile.TileContext(nc) as tc:
        from contextlib import ExitStack

        with ExitStack() as ctx:
            consts = ctx.enter_context(tc.tile_pool(name="consts", bufs=1))
            sb = ctx.enter_context(tc.tile_pool(name="sb", bufs=1))
            ps = ctx.enter_context(tc.tile_pool(name="ps", bufs=1, space="PSUM"))

            # ---- constants ----
            wq = [consts.tile([128, E], bf16, tag=f"wq{c}", name=f"wq{c}") for c in range(NC)]
            wk = [consts.tile([128, E], bf16, tag=f"wk{c}", name=f"wk{c}") for c in range(NC)]
            wv = [consts.tile([128, E], bf16, tag=f"wv{c}", name=f"wv{c}") for c in range(NC)]
            wo = [consts.tile([128, E], bf16, tag=f"wo{c}", name=f"wo{c}") for c in range(NC)]
            for c in range(NC):
                nc.sync.dma_start(wq[c][:], qwT_d[:].rearrange("(t p) e -> t p e", p=128)[c])
                nc.sync.dma_start(wk[c][:], kwT_d[:].rearrange("(t p) e -> t p e", p=128)[c])
                nc.sync.dma_start(wv[c][:], vwT_d[:].rearrange("(t p) e -> t p e", p=128)[c])
                nc.sync.dma_start(wo[c][:], owT_d[:].rearrange("(t p) e -> t p e", p=128)[c])
            tblS = consts.tile([128, T], bf16, tag="tblS")
            nc.sync.dma_start(tblS[:], tblS_d[:])
            tblC = consts.tile([128, T], bf16, tag="tblC")
            nc.sync.dma_start(tblC[:], tblC_d[:])
            pb = consts.tile([128, 8], f32, tag="pb")
            nc.sync.dma_start(pb[:], pb_d[:])
            ob = consts.tile([128, E], f32, tag="ob")
            nc.sync.dma_start(ob[:], ob_d[:])
            mk = consts.tile([128, NT], f32, tag="mk")
            nc.sync.dma_start(mk[:], mask_d[:])
            ident = consts.tile([128, 128], f32r, tag="ident")
            nc.sync.dma_start(ident[:], id_d[:])
            epsc = consts.tile([128, 1], f32, tag="epsc")
            nc.vector.memset(epsc[:], LN_EPS)

            # kbig tiles: [qk-contract layout per head]; table halves loaded once.
            # even head h: rows 0:64 = k, rows 64:128 = [cos;sin] tables
            # odd  head h: rows 0:64 = [cos;sin] tables, rows 64:128 = k
            kbig = [
                sb.tile([128, T], bf16, tag=f"kbig{h}", name=f"kbig{h}")
                for h in range(H)
            ]
            for h in range(H):
                if h % 2 == 0:
                    nc.sync.dma_start(kbig[h][64:128, :], tblK_d[:])
                else:
                    nc.sync.dma_start(kbig[h][0:64, :], tblK_d[:])
            qbig = [
                sb.tile([128, T], bf16, tag=f"qbig{h}", name=f"qbig{h}")
                for h in range(H)
            ]

            # v_ext tiles; masked-ones blocks loaded once.
            # all heads: cols [ones(64) | v(64)] -> AV gives den on PSUM rows
            # 0:64 (so reciprocal_approx_fast runs base-0 aligned; the custom
            # DVE op silently ignores nonzero base partitions) and num on
            # rows 64:128.
            v_ext = [
                sb.tile([128, H * 128], bf16, tag=f"vx{jt}", name=f"vx{jt}")
                for jt in range(NT)
            ]
            for jt in range(NT):
                src = vones_d[:, jt * 512 : (jt + 1) * 512].rearrange(
                    "p (h f) -> p h f", f=64
                )
                dst = v_ext[jt][:].rearrange("p (h s) -> p h s", s=128)
                nc.sync.dma_start(dst[:, :, 0:64], src)

            # ---- phase B: LayerNorm -> z (f32), phase C: transpose -> zT (bf16)
            zT = [sb.tile([128, T], bf16, tag=f"zT{c}", name=f"zT{c}") for c in range(NC)]
            for t in range(NT):
                xt = sb.tile([128, E], f32, tag="x", bufs=2)
                nc.sync.dma_start(xt[:], x_t[t])
                st = sb.tile([128, 6], f32, tag="st", bufs=2)
                nc.vector.bn_stats(st[:], xt[:])
                mv = sb.tile([128, 2], f32, tag="mv", bufs=2)
                nc.vector.bn_aggr(mv[:], st[:])
                sd = sb.tile([128, 1], f32, tag="sd", bufs=2)
                nc.scalar.activation(sd[:], mv[:, 1:2], Act.Sqrt, bias=epsc[:], scale=1.0)
                rstd = sb.tile([128, 1], f32, tag="rstd", bufs=4)
                nc.vector.reciprocal(rstd[:], sd[:])
                nmr = sb.tile([128, 1], f32, tag="nmr", bufs=4)
                nc.vector.scalar_tensor_tensor(
                    nmr[:], mv[:, 0:1], -1.0, rstd[:], Alu.mult, Alu.mult
                )
                zt = sb.tile([128, E], f32r, tag="z", bufs=2)
                nc.vector.tensor_scalar(zt[:], xt[:], rstd[:], nmr[:], Alu.mult, Alu.add)
                pt = ps.tile([128, T], f32, tag="pA", bufs=3)
                for c in range(NC):
                    nc.tensor.transpose(
                        r(pt[:, c * 128 : (c + 1) * 128]),
                        r(zt[:, c * 128 : (c + 1) * 128]),
                        r(ident[:]),
                    )
                for c in range(NC):
                    nc.vector.tensor_copy(
                        zT[c][:, t * 128 : (t + 1) * 128], pt[:, c * 128 : (c + 1) * 128]
                    )

            # ---- phase D: Q/K projections + qbig/kbig build, per head pair ----
            for et in range(NC):
                h0, h1 = 2 * et, 2 * et + 1
                psq = ps.tile([128, T], f32, tag="pA", bufs=3, name=f"psq{et}")
                for ic in range(2):
                    for c in range(NC):
                        nc.tensor.matmul(
                            psq[:, ic * 512 : (ic + 1) * 512],
                            wq[c][:, et * 128 : (et + 1) * 128],
                            zT[c][:, ic * 512 : (ic + 1) * 512],
                            start=(c == 0),
                            stop=(c == NC - 1),
                        )
                psk = ps.tile([128, T], f32, tag="pA", bufs=3, name=f"psk{et}")
                for ic in range(2):
                    for c in range(NC):
                        nc.tensor.matmul(
                            psk[:, ic * 512 : (ic + 1) * 512],
                            wk[c][:, et * 128 : (et + 1) * 128],
                            zT[c][:, ic * 512 : (ic + 1) * 512],
                            start=(c == 0),
                            stop=(c == NC - 1),
                        )
                # q_u rows (scalar engine; idle in this phase)
                nc.scalar.activation(
                    qbig[h0][0:64, :], psq[0:64, :], Act.Identity,
                    bias=pb[0:64, et : et + 1], scale=1.0,
                )
                nc.scalar.activation(
                    qbig[h1][64:128, :], psq[64:128, :], Act.Identity,
                    bias=pb[64:128, et : et + 1], scale=1.0,
                )
                # k rows
                nc.scalar.copy(kbig[h0][0:64, :], psk[0:64, :])
                nc.scalar.copy(kbig[h1][64:128, :], psk[64:128, :])
                # A/B rows: absin (SBUF bf16) and abcos (PSUM f32) products,
                # then mixed-space combines (different base partitions OK).
                absin = sb.tile([128, T], bf16, tag="absin", bufs=2)
                nc.vector.scalar_tensor_tensor(
                    absin[:], psq[:], pb[:, 4 + et : 5 + et], tblS[:], Alu.add, Alu.mult
                )
                abcos = ps.tile([128, T], f32, tag="pA", bufs=3, name=f"abc{et}")
                nc.vector.scalar_tensor_tensor(
                    abcos[:], psq[:], pb[:, 4 + et : 5 + et], tblC[:], Alu.add, Alu.mult
                )
                # A = (qs+b)sin + (qc+b)cos ; B = (qc+b)sin - (qs+b)cos
                nc.vector.tensor_tensor(
                    qbig[h0][64:96, :], absin[0:32, :], abcos[32:64, :], Alu.add
                )
                nc.vector.tensor_tensor(
                    qbig[h0][96:128, :], absin[32:64, :], abcos[0:32, :], Alu.subtract
                )
                nc.vector.tensor_tensor(
                    qbig[h1][0:32, :], absin[64:96, :], abcos[96:128, :], Alu.add
                )
                nc.vector.tensor_tensor(
                    qbig[h1][32:64, :], absin[96:128, :], abcos[64:96, :], Alu.subtract
                )

            # ---- phase V: value projection ----
            for jt in range(NT):
                psv = ps.tile([128, T], f32, tag="pA", bufs=3, name=f"psv{jt}")
                for c in range(NC):
                    nc.tensor.matmul(
                        psv[:, 0:512],
                        zT[c][:, jt * 128 : (jt + 1) * 128],
                        wv[c][:],
                        start=(c == 0),
                        stop=(c == NC - 1),
                    )
                # masked v columns into the second half of each head block
                dst = v_ext[jt][:].rearrange("p (h s) -> p h s", s=128)
                srcv = psv[:, 0:512].rearrange("p (h f) -> p h f", f=64)
                nc.vector.tensor_scalar(
                    dst[:, :, 64:128], srcv, mk[:, jt : jt + 1], None, Alu.mult
                )

            # ---- phase E: attention per head ----
            oT = [
                sb.tile([128, T], bf16, tag=f"oT{m}", name=f"oT{m}") for m in range(4)
            ]
            for h in range(H):
                m = h // 2
                p_tiles = []
                for jt in range(NT):
                    psl = ps.tile([128, T], f32, tag="pA", bufs=3, name=f"psl{h}_{jt}")
                    for ic in range(2):
                        nc.tensor.matmul(
                            psl[:, ic * 512 : (ic + 1) * 512],
                            kbig[h][:, jt * 128 : (jt + 1) * 128],
                            qbig[h][:, ic * 512 : (ic + 1) * 512],
                            start=True,
                            stop=True,
                        )
                    pexp = sb.tile([128, T], bf16, tag="P", bufs=8)
                    p_tiles.append(pexp)
                    nc.scalar.activation(pexp[:], psl[:], Act.Exp, scale=0.125)
                psav = ps.tile([128, T], f32, tag="pB", bufs=1)
                for jt in range(NT):
                    for ic in range(2):
                        nc.tensor.matmul(
                            psav[:, ic * 512 : (ic + 1) * 512],
                            v_ext[jt][:, h * 128 : (h + 1) * 128],
                            p_tiles[jt][:, ic * 512 : (ic + 1) * 512],
                            start=(jt == 0),
                            stop=(jt == NT - 1),
                        )
                # den on rows 0:64 (base-0 aligned for the custom DVE recip),
                # num on rows 64:128; head parity picks the oT pair half.
                rr = sb.tile([64, T], f32, tag="rr", bufs=2)
                nc.vector.reciprocal_approx_fast(rr[:], psav[0:64, :])
                row0 = (h % 2) * 64
                nc.vector.tensor_tensor(
                    oT[m][row0 : row0 + 64, :], psav[64:128, :], rr[:], Alu.mult
                )

            # ---- phase F: output projection ----
            for it in range(NT):
                psy = ps.tile([128, T], f32, tag="pA", bufs=3, name=f"psy{it}")
                for ft in range(NC):
                    nc.tensor.matmul(
                        psy[:, 0:512],
                        oT[ft][:, it * 128 : (it + 1) * 128],
                        wo[ft][:],
                        start=(ft == 0),
                        stop=(ft == NC - 1),
                    )
                yt = sb.tile([128, E], f32, tag="y", bufs=3)
                nc.vector.tensor_add(yt[:], psy[:, 0:512], ob[:])
                nc.sync.dma_start(out_t[it], yt[:])

    if not nc.is_finalized():
        nc.finalize()
    return nc


def _host_prep(inputs):
    """Fold LN gamma/beta + biases into weights; build tables. Returns in_maps."""
    import ml_dtypes

    bf = ml_dtypes.bfloat16
    x = np.asarray(inputs["input_tensor"], np.float32)  # (B, T, E)
    mask = np.asarray(inputs["sequence_mask"])  # (B, T) bool
    gamma = np.asarray(inputs["ln_scale"], np.float32)
    beta = np.asarray(inputs["ln_bias"], np.float32)
    q_w = np.asarray(inputs["q_w"], np.float32)
    k_w = np.asarray(inputs["k_w"], np.float32)
    v_w = np.asarray(inputs["v_w"], np.float32)
    in_b = np.asarray(inputs["in_proj_bias"], np.float32)
    out_w = np.asarray(inputs["out_w"], np.float32)
    out_b = np.asarray(inputs["out_b"], np.float32)
    pos_u = np.asarray(inputs["pos_bias_u"], np.float32).reshape(-1)
    pos_v = np.asarray(inputs["pos_bias_v"], np.float32).reshape(-1)

    bias_k, bias_q, bias_v = np.split(in_b, 3)  # torch unpack order

    qw_eff = q_w * gamma[None, :]
    kw_eff = k_w * gamma[None, :]
    vw_eff = v_w * gamma[None, :]
    bias_qu = q_w @ beta + bias_q + pos_u
    bias_qv = q_w @ beta + bias_q + pos_v
    bias_v_full = v_w @ beta + bias_v
    out_b_eff = out_b + out_w @ bias_v_full

    qwT = np.ascontiguousarray(qw_eff.T).astype(bf)
    kwT = np.ascontiguousarray(kw_eff.T).astype(bf)
    vwT = np.ascontiguousarray(vw_eff.T).astype(bf)
    owT = np.ascontiguousarray(out_w.T).astype(bf)

    pbias = np.zeros((128, 8), np.float32)
    for et in range(NC):
        pbias[:, et] = bias_qu[et * 128 : (et + 1) * 128]
        pbias[:, 4 + et] = bias_qv[et * 128 : (et + 1) * 128]

    inv_freq = (1.0 / (10000.0 ** (np.arange(0, DH, 2, dtype=np.float64) / DH))).astype(
        np.float64
    )
    ang = np.outer(inv_freq, np.arange(T, dtype=np.float64))  # (32, T)
    cosT = np.cos(ang)
    sinT = np.sin(ang)
    tblk = np.concatenate([cosT, sinT], axis=0).astype(bf)  # (64, T)
    tblsin = np.tile(sinT, (4, 1)).astype(bf)  # (128, T)
    tblcos = np.tile(cosT, (4, 1)).astype(bf)  # (128, T)

    obias = np.tile(out_b_eff.reshape(1, E), (128, 1)).astype(np.float32)
    ident = np.eye(128, dtype=np.float32)

    shared = {
        "qwT": qwT,
        "kwT": kwT,
        "vwT": vwT,
        "owT": owT,
        "pbias": pbias,
        "tblk": tblk,
        "tblsin": tblsin,
        "tblcos": tblcos,
        "obias": obias,
        "ident": ident,
    }
    in_maps = []
    for b in range(N_CORES):
        mt = np.zeros((128, NT), np.float32)
        mb = mask[b].astype(np.float32)
        for jt in range(NT):
            mt[:, jt] = mb[jt * 128 : (jt + 1) * 128]
        # vones: per jt an 8x-repeated [128, 64] masked-ones block
        vones = np.repeat(mt.T.reshape(NT, 128, 1), 512, axis=2)  # (NT,128,512)
        vones = np.ascontiguousarray(
            vones.transpose(1, 0, 2).reshape(128, NT * 512)
        ).astype(bf)
        in_maps.append(
            {"x": np.ascontiguousarray(x[b]), "maskt": mt, "vones": vones, **shared}
        )
    return in_maps


def kernel(**inputs) -> np.ndarray:
    from concourse.bass_utils import run_bass_kernel_spmd

    in_maps = _host_prep(inputs)
    if "nc" not in _CACHE:
        _CACHE["nc"] = _build_nc()
    trace = os.environ.get("KERNEL_TRACE", "0") == "1"
    res = run_bass_kernel_spmd(
        _CACHE["nc"], in_maps, core_ids=list(range(N_CORES)), trace=trace
    )
    _CACHE["last_result"] = res
    out = np.stack([res.results[i]["out"] for i in range(N_CORES)], axis=0)
    return out.astype(np.float32)


# revision 9
# speedup vs baseline: 1.7745x; 1.1781x over previous
"""Trainium2 Bass kernel: Conformer MHSA with relative positional encoding.

Shapes (hardcoded): B=8, T=1024, E=512, H=8, DH=64.
Sharding: data-parallel over batch -- one batch element per NeuronCore (8 cores).

v4. Structure (all matmuls bf16; accuracy gate rel<2e-2, measured ~6e-3):
  - LayerNorm gamma/beta folded into projection weights/biases on host.
  - Rel-pos term folded into the logits matmul via angle-addition
    (contraction 64 -> 128); the rel-shift becomes implicit.
  - k bias dropped (softmax-invariant); v bias folded into output bias.
  - Softmax denominator rides the AV matmul as masked ones-columns; v_ext
    column order [ones|v] for every head puts den on PSUM rows 0:64 (the
    custom-DVE reciprocal_approx_fast only works at base partition 0) and
    num on rows 64:128, which lands odd heads' output in place for the
    paired oT tile.  Cross-partition moves use engine copies / mixed-space
    (SBUF+PSUM) tensor_tensor operands -- no shift DMAs.
  - x DMAs issue first so LayerNorm starts immediately; ones-columns are
    built on-chip (memset+mask-mul) instead of a descriptor-heavy DMA.
  - V-projection matmuls are interleaved into the Q/K phase to keep the PE
    dense while the vector engine builds qbig.
  - Attention is software-pipelined across heads: AV(h-1, 4..7) interleaves
    with logits(h, 0..3) so the PE never waits for the scalar engine's exp
    queue to drain at a head boundary.
"""

import os
import sys

import numpy as np

sys.path.insert(0, "/opt/trn_rl_repo")

T = 1024
E = 512
H = 8
DH = 64
NT = T // 128  # 8 i/j tiles
NC = E // 128  # 4 c/e tiles
LN_EPS = 1e-5
N_CORES = 8

_CACHE = {}


def _build_nc():
    import concourse.bass as bass
    import concourse.tile as tile
    from concourse import bacc, mybir

    f32 = mybir.dt.float32
    f32r = mybir.dt.float32r
    bf16 = mybir.dt.bfloat16
    Alu = mybir.AluOpType
    Act = mybir.ActivationFunctionType

    def r(ap):
        return ap.bitcast(f32r)

    nc = bacc.Bacc("TRN2", target_bir_lowering=False, debug=False, num_devices=N_CORES)

    x_d = nc.declare_dram_parameter("x", [T, E], f32, isOutput=False)
    qwT_d = nc.declare_dram_parameter("qwT", [E, E], bf16, isOutput=False)
    kwT_d = nc.declare_dram_parameter("kwT", [E, E], bf16, isOutput=False)
    vwT_d = nc.declare_dram_parameter("vwT", [E, E], bf16, isOutput=False)
    owT_d = nc.declare_dram_parameter("owT", [E, E], bf16, isOutput=False)
    pb_d = nc.declare_dram_parameter("pbias", [128, 8], f32, isOutput=False)
    tblS_d = nc.declare_dram_parameter("tblsin", [128, T], bf16, isOutput=False)
    tblC_d = nc.declare_dram_parameter("tblcos", [128, T], bf16, isOutput=False)
    tblK_d = nc.declare_dram_parameter("tblk", [DH, T], bf16, isOutput=False)
    ob_d = nc.declare_dram_parameter("obias", [128, E], f32, isOutput=False)
    mask_d = nc.declare_dram_parameter("maskt", [128, NT], f32, isOutput=False)
    id_d = nc.declare_dram_parameter("ident", [128, 128], f32r, isOutput=False)
    out_d = nc.declare_dram_parameter("out", [T, E], f32, isOutput=True)

    x_t = x_d[:].rearrange("(t p) e -> t p e", p=128)
    out_t = out_d[:].rearrange("(t p) e -> t p e", p=128)

    with tile.TileContext(nc) as tc:
        from contextlib import ExitStack

        with ExitStack() as ctx:
            consts = ctx.enter_context(tc.tile_pool(name="consts", bufs=1))
            sb = ctx.enter_context(tc.tile_pool(name="sb", bufs=1))
            ps = ctx.enter_context(tc.tile_pool(name="ps", bufs=1, space="PSUM"))

            # ---- x first: LayerNorm depends on it, everything else can wait
            x_tiles = []
            for t in range(NT):
                xt = sb.tile([128, E], f32, tag="x", bufs=8, name=f"x{t}")
                nc.sync.dma_start(xt[:], x_t[t])
                x_tiles.append(xt)

            # ---- constants ----
            ident = consts.tile([128, 128], f32r, tag="ident")
            nc.sync.dma_start(ident[:], id_d[:])
            pb = consts.tile([128, 8], f32, tag="pb")
            nc.sync.dma_start(pb[:], pb_d[:])
            mk = consts.tile([128, NT], f32, tag="mk")
            nc.sync.dma_start(mk[:], mask_d[:])
            wq = [consts.tile([128, E], bf16, tag=f"wq{c}", name=f"wq{c}") for c in range(NC)]
            wk = [consts.tile([128, E], bf16, tag=f"wk{c}", name=f"wk{c}") for c in range(NC)]
            wv = [consts.tile([128, E], bf16, tag=f"wv{c}", name=f"wv{c}") for c in range(NC)]
            wo = [consts.tile([128, E], bf16, tag=f"wo{c}", name=f"wo{c}") for c in range(NC)]
            for c in range(NC):
                nc.sync.dma_start(wq[c][:], qwT_d[:].rearrange("(t p) e -> t p e", p=128)[c])
                nc.sync.dma_start(wk[c][:], kwT_d[:].rearrange("(t p) e -> t p e", p=128)[c])
                nc.sync.dma_start(wv[c][:], vwT_d[:].rearrange("(t p) e -> t p e", p=128)[c])
                nc.sync.dma_start(wo[c][:], owT_d[:].rearrange("(t p) e -> t p e", p=128)[c])
            tblS = consts.tile([128, T], bf16, tag="tblS")
            nc.sync.dma_start(tblS[:], tblS_d[:])
            tblC = consts.tile([128, T], bf16, tag="tblC")
            nc.sync.dma_start(tblC[:], tblC_d[:])
            ob = consts.tile([128, E], f32, tag="ob")
            nc.sync.dma_start(ob[:], ob_d[:])
            epsc = consts.tile([128, 1], f32, tag="epsc")
            nc.vector.memset(epsc[:], LN_EPS)

            # kbig: even head h: rows 0:64 = k, 64:128 = [cos;sin];
            #       odd  head h: rows 0:64 = [cos;sin], 64:128 = k.
            kbig = [
                sb.tile([128, T], bf16, tag=f"kbig{h}", name=f"kbig{h}")
                for h in range(H)
            ]
            for h in range(H):
                if h % 2 == 0:
                    nc.sync.dma_start(kbig[h][64:128, :], tblK_d[:])
                else:
                    nc.sync.dma_start(kbig[h][0:64, :], tblK_d[:])
            qbig = [
                sb.tile([128, T], bf16, tag=f"qbig{h}", name=f"qbig{h}")
                for h in range(H)
            ]

            # v_ext: all heads [ones(64) | v(64)] -> den on PSUM rows 0:64.
            # ones built on-chip: memset + per-partition mask multiply.
            v_ext = [
                sb.tile([128, H * 128], bf16, tag=f"vx{jt}", name=f"vx{jt}")
                for jt in range(NT)
            ]
            for jt in range(NT):
                ones_view = v_ext[jt][:].rearrange("p (h s) -> p h s", s=128)[:, :, 0:64]
                nc.gpsimd.memset(ones_view, 1.0)
                nc.gpsimd.tensor_scalar(
                    ones_view, ones_view, mk[:, jt : jt + 1], None, Alu.mult
                )

            # ---- phase B/C: LayerNorm + transpose -> zT (bf16)
            zT = [sb.tile([128, T], bf16, tag=f"zT{c}", name=f"zT{c}") for c in range(NC)]
            for t in range(NT):
                xt = x_tiles[t]
                st = sb.tile([128, 6], f32, tag="st", bufs=2)
                nc.vector.bn_stats(st[:], xt[:])
                mv = sb.tile([128, 2], f32, tag="mv", bufs=2)
                nc.vector.bn_aggr(mv[:], st[:])
                sd = sb.tile([128, 1], f32, tag="sd", bufs=2)
                nc.scalar.activation(sd[:], mv[:, 1:2], Act.Sqrt, bias=epsc[:], scale=1.0)
                rstd = sb.tile([128, 1], f32, tag="rstd", bufs=4)
                nc.vector.reciprocal(rstd[:], sd[:])
                nmr = sb.tile([128, 1], f32, tag="nmr", bufs=4)
                nc.vector.scalar_tensor_tensor(
                    nmr[:], mv[:, 0:1], -1.0, rstd[:], Alu.mult, Alu.mult
                )
                zt = sb.tile([128, E], f32r, tag="z", bufs=2)
                nc.scalar.activation(zt[:], xt[:], Act.Identity, bias=nmr[:], scale=rstd[:])
                pt = ps.tile([128, T], f32, tag="pA", bufs=3)
                for c in range(NC):
                    nc.tensor.transpose(
                        r(pt[:, c * 128 : (c + 1) * 128]),
                        r(zt[:, c * 128 : (c + 1) * 128]),
                        r(ident[:]),
                    )
                for c in range(NC):
                    eng = nc.scalar.copy if c < 2 else nc.vector.tensor_copy
                    eng(zT[c][:, t * 128 : (t + 1) * 128], pt[:, c * 128 : (c + 1) * 128])

            # ---- phase D: Q/K/V projections + qbig build, per head pair ----
            for et in range(NC):
                h0, h1 = 2 * et, 2 * et + 1
                psq = ps.tile([128, T], f32, tag="pA", bufs=3, name=f"psq{et}")
                for ic in range(2):
                    for c in range(NC):
                        nc.tensor.matmul(
                            psq[:, ic * 512 : (ic + 1) * 512],
                            wq[c][:, et * 128 : (et + 1) * 128],
                            zT[c][:, ic * 512 : (ic + 1) * 512],
                            start=(c == 0),
                            stop=(c == NC - 1),
                        )
                psk = ps.tile([128, T], f32, tag="pA", bufs=3, name=f"psk{et}")
                for ic in range(2):
                    for c in range(NC):
                        nc.tensor.matmul(
                            psk[:, ic * 512 : (ic + 1) * 512],
                            wk[c][:, et * 128 : (et + 1) * 128],
                            zT[c][:, ic * 512 : (ic + 1) * 512],
                            start=(c == 0),
                            stop=(c == NC - 1),
                        )
                # V for two j-tiles, keeps the PE dense while DVE builds qbig
                for jt in (2 * et, 2 * et + 1):
                    psv = ps.tile([128, T], f32, tag="pB", bufs=1, name=f"psv{jt}")
                    for c in range(NC):
                        nc.tensor.matmul(
                            psv[:, 0:512],
                            zT[c][:, jt * 128 : (jt + 1) * 128],
                            wv[c][:],
                            start=(c == 0),
                            stop=(c == NC - 1),
                        )
                    dst = v_ext[jt][:].rearrange("p (h s) -> p h s", s=128)
                    srcv = psv[:, 0:512].rearrange("p (h f) -> p h f", f=64)
                    nc.vector.tensor_scalar(
                        dst[:, :, 64:128], srcv, mk[:, jt : jt + 1], None, Alu.mult
                    )
                # q_u rows (scalar engine)
                nc.scalar.activation(
                    qbig[h0][0:64, :], psq[0:64, :], Act.Identity,
                    bias=pb[0:64, et : et + 1], scale=1.0,
                )
                nc.scalar.activation(
                    qbig[h1][64:128, :], psq[64:128, :], Act.Identity,
                    bias=pb[64:128, et : et + 1], scale=1.0,
                )
                # k rows
                nc.scalar.copy(kbig[h0][0:64, :], psk[0:64, :])
                nc.scalar.copy(kbig[h1][64:128, :], psk[64:128, :])
                # A/B rows via absin (SBUF bf16) and abcos (PSUM f32) products
                absin = sb.tile([128, T], bf16, tag="absin", bufs=2)
                nc.vector.scalar_tensor_tensor(
                    absin[:], psq[:], pb[:, 4 + et : 5 + et], tblS[:], Alu.add, Alu.mult
                )
                abcos = ps.tile([128, T], f32, tag="pA", bufs=3, name=f"abc{et}")
                nc.vector.scalar_tensor_tensor(
                    abcos[:], psq[:], pb[:, 4 + et : 5 + et], tblC[:], Alu.add, Alu.mult
                )
                # A = (qs+b)sin + (qc+b)cos ; B = (qc+b)sin - (qs+b)cos
                nc.vector.tensor_tensor(
                    qbig[h0][64:96, :], absin[0:32, :], abcos[32:64, :], Alu.add
                )
                nc.vector.tensor_tensor(
                    qbig[h0][96:128, :], absin[32:64, :], abcos[0:32, :], Alu.subtract
                )
                nc.vector.tensor_tensor(
                    qbig[h1][0:32, :], absin[64:96, :], abcos[96:128, :], Alu.add
                )
                nc.vector.tensor_tensor(
                    qbig[h1][32:64, :], absin[96:128, :], abcos[64:96, :], Alu.subtract
                )

            # ---- phase E: attention, software-pipelined across heads ----
            oT = [
                sb.tile([128, T], bf16, tag=f"oT{m}", name=f"oT{m}") for m in range(4)
            ]
            p_tiles = {}
            psav_t = {}

            def emit_logit(h, jt):
                psl = ps.tile([128, T], f32, tag="pA", bufs=3, name=f"psl{h}_{jt}")
                for ic in range(2):
                    nc.tensor.matmul(
                        psl[:, ic * 512 : (ic + 1) * 512],
                        kbig[h][:, jt * 128 : (jt + 1) * 128],
                        qbig[h][:, ic * 512 : (ic + 1) * 512],
                        start=True,
                        stop=True,
                    )
                pexp = sb.tile([128, T], bf16, tag="P", bufs=12)
                p_tiles[(h, jt)] = pexp
                nc.scalar.activation(pexp[:], psl[:], Act.Exp, scale=0.125)

            def emit_av(h, jt):
                if jt == 0:
                    psav_t[h] = ps.tile([128, T], f32, tag="pB", bufs=1, name=f"psav{h}")
                psav = psav_t[h]
                for ic in range(2):
                    nc.tensor.matmul(
                        psav[:, ic * 512 : (ic + 1) * 512],
                        v_ext[jt][:, h * 128 : (h + 1) * 128],
                        p_tiles[(h, jt)][:, ic * 512 : (ic + 1) * 512],
                        start=(jt == 0),
                        stop=(jt == NT - 1),
                    )

            def emit_div(h):
                psav = psav_t[h]
                m = h // 2
                rr = sb.tile([64, T], f32, tag="rr", bufs=2)
                nc.vector.reciprocal_approx_fast(rr[:], psav[0:64, :])
                row0 = (h % 2) * 64
                nc.vector.tensor_tensor(
                    oT[m][row0 : row0 + 64, :], psav[64:128, :], rr[:], Alu.mult
                )

            for h in range(H):
                for jt in range(4):
                    emit_logit(h, jt)
                    if h > 0:
                        emit_av(h - 1, 4 + jt)
                if h > 0:
                    emit_div(h - 1)
                for jt in range(4, 8):
                    emit_logit(h, jt)
                for jt in range(4):
                    emit_av(h, jt)
            for jt in range(4, 8):
                emit_av(H - 1, jt)
            emit_div(H - 1)

            # ---- phase F: output projection ----
            for it in range(NT):
                psy = ps.tile([128, T], f32, tag="pA", bufs=3, name=f"psy{it}")
                for ft in range(NC):
                    nc.tensor.matmul(
                        psy[:, 0:512],
                        oT[ft][:, it * 128 : (it + 1) * 128],
                        wo[ft][:],
                        start=(ft == 0),
                        stop=(ft == NC - 1),
                    )
                yt = sb.tile([128, E], f32, tag="y", bufs=3)
                nc.vector.tensor_add(yt[:], psy[:, 0:512], ob[:])
                nc.sync.dma_start(out_t[it], yt[:])

    if not nc.is_finalized():
        nc.finalize()
    return nc


def _host_prep(inputs):
    """Fold LN gamma/beta + biases into weights; build tables. Returns in_maps."""
    import ml_dtypes

    bf = ml_dtypes.bfloat16
    x = np.asarray(inputs["input_tensor"], np.float32)  # (B, T, E)
    mask = np.asarray(inputs["sequence_mask"])  # (B, T) bool
    gamma = np.asarray(inputs["ln_scale"], np.float32)
    beta = np.asarray(inputs["ln_bias"], np.float32)
    q_w = np.asarray(inputs["q_w"], np.float32)
    k_w = np.asarray(inputs["k_w"], np.float32)
    v_w = np.asarray(inputs["v_w"], np.float32)
    in_b = np.asarray(inputs["in_proj_bias"], np.float32)
    out_w = np.asarray(inputs["out_w"], np.float32)
    out_b = np.asarray(inputs["out_b"], np.float32)
    pos_u = np.asarray(inputs["pos_bias_u"], np.float32).reshape(-1)
    pos_v = np.asarray(inputs["pos_bias_v"], np.float32).reshape(-1)

    bias_k, bias_q, bias_v = np.split(in_b, 3)  # torch unpack order

    qw_eff = q_w * gamma[None, :]
    kw_eff = k_w * gamma[None, :]
    vw_eff = v_w * gamma[None, :]
    bias_qu = q_w @ beta + bias_q + pos_u
    bias_qv = q_w @ beta + bias_q + pos_v
    bias_v_full = v_w @ beta + bias_v
    out_b_eff = out_b + out_w @ bias_v_full

    qwT = np.ascontiguousarray(qw_eff.T).astype(bf)
    kwT = np.ascontiguousarray(kw_eff.T).astype(bf)
    vwT = np.ascontiguousarray(vw_eff.T).astype(bf)
    owT = np.ascontiguousarray(out_w.T).astype(bf)

    pbias = np.zeros((128, 8), np.float32)
    for et in range(NC):
        pbias[:, et] = bias_qu[et * 128 : (et + 1) * 128]
        pbias[:, 4 + et] = bias_qv[et * 128 : (et + 1) * 128]

    inv_freq = (1.0 / (10000.0 ** (np.arange(0, DH, 2, dtype=np.float64) / DH))).astype(
        np.float64
    )
    ang = np.outer(inv_freq, np.arange(T, dtype=np.float64))  # (32, T)
    cosT = np.cos(ang)
    sinT = np.sin(ang)
    tblk = np.concatenate([cosT, sinT], axis=0).astype(bf)  # (64, T)
    tblsin = np.tile(sinT, (4, 1)).astype(bf)  # (128, T)
    tblcos = np.tile(cosT, (4, 1)).astype(bf)  # (128, T)

    obias = np.tile(out_b_eff.reshape(1, E), (128, 1)).astype(np.float32)
    ident = np.eye(128, dtype=np.float32)

    shared = {
        "qwT": qwT,
        "kwT": kwT,
        "vwT": vwT,
        "owT": owT,
        "pbias": pbias,
        "tblk": tblk,
        "tblsin": tblsin,
        "tblcos": tblcos,
        "obias": obias,
        "ident": ident,
    }
    in_maps = []
    for b in range(N_CORES):
        mt = np.zeros((128, NT), np.float32)
        mb = mask[b].astype(np.float32)
        for jt in range(NT):
            mt[:, jt] = mb[jt * 128 : (jt + 1) * 128]
        in_maps.append({"x": np.ascontiguousarray(x[b]), "maskt": mt, **shared})
    return in_maps


def kernel(**inputs) -> np.ndarray:
    from concourse.bass_utils import run_bass_kernel_spmd

    in_maps = _host_prep(inputs)
    if "nc" not in _CACHE:
        _CACHE["nc"] = _build_nc()
    trace = os.environ.get("KERNEL_TRACE", "0") == "1"
    res = run_bass_kernel_spmd(
        _CACHE["nc"], in_maps, core_ids=list(range(N_CORES)), trace=trace
    )
    _CACHE["last_result"] = res
    out = np.stack([res.results[i]["out"] for i in range(N_CORES)], axis=0)
    return out.astype(np.float32)


# revision 10
# speedup vs baseline: 1.9634x; 1.1064x over previous
"""Trainium2 Bass kernel: Conformer MHSA with relative positional encoding.

Shapes (hardcoded): B=8, T=1024, E=512, H=8, DH=64.
Sharding: data-parallel over batch -- one batch element per NeuronCore (8 cores).

v5. Structure (all matmuls bf16; accuracy gate rel<2e-2, measured ~6e-3):
  - LayerNorm gamma/beta folded into projection weights/biases on host.
  - Rel-pos term folded into the logits matmul via angle-addition
    (contraction 64 -> 128); the rel-shift becomes implicit.
  - k bias dropped (softmax-invariant); v bias folded into output bias.
  - Softmax denominator rides the AV matmul as masked ones-columns; v_ext
    column order [ones|v] for every head puts den on PSUM rows 0:64 (the
    custom-DVE reciprocal_approx_fast only works at base partition 0) and
    num on rows 64:128, which lands odd heads' output in place for the
    paired oT tile.  Cross-partition moves use engine copies / mixed-space
    (SBUF+PSUM) tensor_tensor operands -- no shift DMAs.
  - Attention is software-pipelined across heads (AV(h-1) interleaves with
    logits(h)), and the Q/K projections + qbig builds for head pairs 1..3
    are embedded INSIDE the attention stream: the PE takes the extra
    matmuls in its slack, the vector engine builds qbig in its slack, and
    the scalar engine stays 100% on exp (the true critical resource).
  - abcos is computed in place over psq (PSUM) so an embedded build needs
    only one extra PSUM tile.
"""

import os
import sys

import numpy as np

sys.path.insert(0, "/opt/trn_rl_repo")

T = 1024
E = 512
H = 8
DH = 64
NT = T // 128  # 8 i/j tiles
NC = E // 128  # 4 c/e tiles
LN_EPS = 1e-5
N_CORES = 8

_CACHE = {}


def _build_nc():
    import concourse.bass as bass
    import concourse.tile as tile
    from concourse import bacc, mybir

    f32 = mybir.dt.float32
    f32r = mybir.dt.float32r
    bf16 = mybir.dt.bfloat16
    Alu = mybir.AluOpType
    Act = mybir.ActivationFunctionType

    def r(ap):
        return ap.bitcast(f32r)

    nc = bacc.Bacc("TRN2", target_bir_lowering=False, debug=False, num_devices=N_CORES)

    x_d = nc.declare_dram_parameter("x", [T, E], f32, isOutput=False)
    qwT_d = nc.declare_dram_parameter("qwT", [E, E], bf16, isOutput=False)
    kwT_d = nc.declare_dram_parameter("kwT", [E, E], bf16, isOutput=False)
    vwT_d = nc.declare_dram_parameter("vwT", [E, E], bf16, isOutput=False)
    owT_d = nc.declare_dram_parameter("owT", [E, E], bf16, isOutput=False)
    pb_d = nc.declare_dram_parameter("pbias", [128, 8], f32, isOutput=False)
    tblS_d = nc.declare_dram_parameter("tblsin", [128, T], bf16, isOutput=False)
    tblC_d = nc.declare_dram_parameter("tblcos", [128, T], bf16, isOutput=False)
    tblK_d = nc.declare_dram_parameter("tblk", [DH, T], bf16, isOutput=False)
    ob_d = nc.declare_dram_parameter("obias", [128, E], f32, isOutput=False)
    mask_d = nc.declare_dram_parameter("maskt", [128, NT], f32, isOutput=False)
    vones_d = nc.declare_dram_parameter("vones", [128, NT * 512], bf16, isOutput=False)
    id_d = nc.declare_dram_parameter("ident", [128, 128], f32r, isOutput=False)
    out_d = nc.declare_dram_parameter("out", [T, E], f32, isOutput=True)

    x_t = x_d[:].rearrange("(t p) e -> t p e", p=128)
    out_t = out_d[:].rearrange("(t p) e -> t p e", p=128)

    with tile.TileContext(nc) as tc:
        from contextlib import ExitStack

        with ExitStack() as ctx:
            consts = ctx.enter_context(tc.tile_pool(name="consts", bufs=1))
            sb = ctx.enter_context(tc.tile_pool(name="sb", bufs=1))
            ps = ctx.enter_context(tc.tile_pool(name="ps", bufs=1, space="PSUM"))

            # ---- x first: LayerNorm depends on it ----
            x_tiles = []
            for t in range(NT):
                xt = sb.tile([128, E], f32, tag="x", bufs=8, name=f"x{t}")
                nc.sync.dma_start(xt[:], x_t[t])
                x_tiles.append(xt)

            # ---- constants ----
            ident = consts.tile([128, 128], f32r, tag="ident")
            nc.sync.dma_start(ident[:], id_d[:])
            pb = consts.tile([128, 8], f32, tag="pb")
            nc.sync.dma_start(pb[:], pb_d[:])
            mk = consts.tile([128, NT], f32, tag="mk")
            nc.sync.dma_start(mk[:], mask_d[:])
            wq = [consts.tile([128, E], bf16, tag=f"wq{c}", name=f"wq{c}") for c in range(NC)]
            wk = [consts.tile([128, E], bf16, tag=f"wk{c}", name=f"wk{c}") for c in range(NC)]
            wv = [consts.tile([128, E], bf16, tag=f"wv{c}", name=f"wv{c}") for c in range(NC)]
            wo = [consts.tile([128, E], bf16, tag=f"wo{c}", name=f"wo{c}") for c in range(NC)]
            for c in range(NC):
                nc.sync.dma_start(wq[c][:], qwT_d[:].rearrange("(t p) e -> t p e", p=128)[c])
                nc.sync.dma_start(wk[c][:], kwT_d[:].rearrange("(t p) e -> t p e", p=128)[c])
                nc.sync.dma_start(wv[c][:], vwT_d[:].rearrange("(t p) e -> t p e", p=128)[c])
                nc.sync.dma_start(wo[c][:], owT_d[:].rearrange("(t p) e -> t p e", p=128)[c])
            tblS = consts.tile([128, T], bf16, tag="tblS")
            nc.sync.dma_start(tblS[:], tblS_d[:])
            tblC = consts.tile([128, T], bf16, tag="tblC")
            nc.sync.dma_start(tblC[:], tblC_d[:])
            ob = consts.tile([128, E], f32, tag="ob")
            nc.sync.dma_start(ob[:], ob_d[:])
            epsc = consts.tile([128, 1], f32, tag="epsc")
            nc.vector.memset(epsc[:], LN_EPS)

            # kbig: even head h: rows 0:64 = k, 64:128 = [cos;sin];
            #       odd  head h: rows 0:64 = [cos;sin], 64:128 = k.
            kbig = [
                sb.tile([128, T], bf16, tag=f"kbig{h}", name=f"kbig{h}")
                for h in range(H)
            ]
            for h in range(H):
                if h % 2 == 0:
                    nc.sync.dma_start(kbig[h][64:128, :], tblK_d[:])
                else:
                    nc.sync.dma_start(kbig[h][0:64, :], tblK_d[:])
            qbig = [
                sb.tile([128, T], bf16, tag=f"qbig{h}", name=f"qbig{h}")
                for h in range(H)
            ]

            # v_ext: all heads [ones(64) | v(64)] -> den on PSUM rows 0:64.
            v_ext = [
                sb.tile([128, H * 128], bf16, tag=f"vx{jt}", name=f"vx{jt}")
                for jt in range(NT)
            ]
            for jt in range(NT):
                src = vones_d[:, jt * 512 : (jt + 1) * 512].rearrange(
                    "p (h f) -> p h f", f=64
                )
                dst = v_ext[jt][:].rearrange("p (h s) -> p h s", s=128)
                nc.sync.dma_start(dst[:, :, 0:64], src)

            # ---- phase B/C: LayerNorm + transpose -> zT (bf16)
            zT = [sb.tile([128, T], bf16, tag=f"zT{c}", name=f"zT{c}") for c in range(NC)]
            for t in range(NT):
                xt = x_tiles[t]
                st = sb.tile([128, 6], f32, tag="st", bufs=2)
                nc.vector.bn_stats(st[:], xt[:])
                mv = sb.tile([128, 2], f32, tag="mv", bufs=2)
                nc.vector.bn_aggr(mv[:], st[:])
                sd = sb.tile([128, 1], f32, tag="sd", bufs=2)
                nc.scalar.activation(sd[:], mv[:, 1:2], Act.Sqrt, bias=epsc[:], scale=1.0)
                rstd = sb.tile([128, 1], f32, tag="rstd", bufs=4)
                nc.vector.reciprocal(rstd[:], sd[:])
                nmr = sb.tile([128, 1], f32, tag="nmr", bufs=4)
                nc.vector.scalar_tensor_tensor(
                    nmr[:], mv[:, 0:1], -1.0, rstd[:], Alu.mult, Alu.mult
                )
                zt = sb.tile([128, E], f32r, tag="z", bufs=2)
                nc.scalar.activation(zt[:], xt[:], Act.Identity, bias=nmr[:], scale=rstd[:])
                pt = ps.tile([128, T], f32, tag="pA", bufs=2)
                for c in range(NC):
                    nc.tensor.transpose(
                        r(pt[:, c * 128 : (c + 1) * 128]),
                        r(zt[:, c * 128 : (c + 1) * 128]),
                        r(ident[:]),
                    )
                for c in range(NC):
                    eng = nc.scalar.copy if c < 2 else nc.vector.tensor_copy
                    eng(zT[c][:, t * 128 : (t + 1) * 128], pt[:, c * 128 : (c + 1) * 128])

            # ---- phase V: value projection (PE-dense, fills DVE-light gap)
            for jt in range(NT):
                psv = ps.tile([128, T], f32, tag="pA", bufs=2, name=f"psv{jt}")
                for c in range(NC):
                    nc.tensor.matmul(
                        psv[:, 0:512],
                        zT[c][:, jt * 128 : (jt + 1) * 128],
                        wv[c][:],
                        start=(c == 0),
                        stop=(c == NC - 1),
                    )
                dst = v_ext[jt][:].rearrange("p (h s) -> p h s", s=128)
                srcv = psv[:, 0:512].rearrange("p (h f) -> p h f", f=64)
                nc.vector.tensor_scalar(
                    dst[:, :, 64:128], srcv, mk[:, jt : jt + 1], None, Alu.mult
                )

            # ---- Q/K projection + qbig/kbig build helpers ----
            psq_t = {}

            def emit_k_mm(et):
                psk = ps.tile([128, T], f32, tag="pC", bufs=1, name=f"psk{et}")
                for ic in range(2):
                    for c in range(NC):
                        nc.tensor.matmul(
                            psk[:, ic * 512 : (ic + 1) * 512],
                            wk[c][:, et * 128 : (et + 1) * 128],
                            zT[c][:, ic * 512 : (ic + 1) * 512],
                            start=(c == 0),
                            stop=(c == NC - 1),
                        )
                return psk

            def emit_q_mm(et):
                psq = ps.tile([128, T], f32, tag="pC", bufs=1, name=f"psq{et}")
                psq_t[et] = psq
                for ic in range(2):
                    for c in range(NC):
                        nc.tensor.matmul(
                            psq[:, ic * 512 : (ic + 1) * 512],
                            wq[c][:, et * 128 : (et + 1) * 128],
                            zT[c][:, ic * 512 : (ic + 1) * 512],
                            start=(c == 0),
                            stop=(c == NC - 1),
                        )

            def emit_kevac(psk, et, scalar_eng):
                h0, h1 = 2 * et, 2 * et + 1
                if scalar_eng:
                    nc.scalar.copy(kbig[h0][0:64, :], psk[0:64, :])
                    nc.scalar.copy(kbig[h1][64:128, :], psk[64:128, :])
                else:
                    nc.vector.tensor_copy(kbig[h0][0:64, :], psk[0:64, :])
                    nc.vector.tensor_copy(kbig[h1][64:128, :], psk[64:128, :])

            def emit_qbuild(et, scalar_eng):
                """q_u rows, then absin, then abcos IN PLACE over psq, then
                the A/B combines.  All VE (or scalar for q_u in prologue)."""
                h0, h1 = 2 * et, 2 * et + 1
                psq = psq_t[et]
                if scalar_eng:
                    nc.scalar.activation(
                        qbig[h0][0:64, :], psq[0:64, :], Act.Identity,
                        bias=pb[0:64, et : et + 1], scale=1.0,
                    )
                    nc.scalar.activation(
                        qbig[h1][64:128, :], psq[64:128, :], Act.Identity,
                        bias=pb[64:128, et : et + 1], scale=1.0,
                    )
                else:
                    nc.vector.tensor_scalar(
                        qbig[h0][0:64, :], psq[0:64, :], pb[0:64, et : et + 1],
                        None, Alu.add,
                    )
                    nc.vector.tensor_scalar(
                        qbig[h1][64:128, :], psq[64:128, :], pb[64:128, et : et + 1],
                        None, Alu.add,
                    )
                absin = sb.tile([128, T], bf16, tag="absin", bufs=2)
                nc.vector.scalar_tensor_tensor(
                    absin[:], psq[:], pb[:, 4 + et : 5 + et], tblS[:], Alu.add, Alu.mult
                )
                # abcos overwrites psq in place (last reader of the raw psq)
                nc.vector.scalar_tensor_tensor(
                    psq[:], psq[:], pb[:, 4 + et : 5 + et], tblC[:], Alu.add, Alu.mult
                )
                # A = (qs+b)sin + (qc+b)cos ; B = (qc+b)sin - (qs+b)cos
                nc.vector.tensor_tensor(
                    qbig[h0][64:96, :], absin[0:32, :], psq[32:64, :], Alu.add
                )
                nc.vector.tensor_tensor(
                    qbig[h0][96:128, :], absin[32:64, :], psq[0:32, :], Alu.subtract
                )
                nc.vector.tensor_tensor(
                    qbig[h1][0:32, :], absin[64:96, :], psq[96:128, :], Alu.add
                )
                nc.vector.tensor_tensor(
                    qbig[h1][32:64, :], absin[96:128, :], psq[64:96, :], Alu.subtract
                )

            # et0 in the prologue (scalar engine is still free here)
            psk0 = emit_k_mm(0)
            emit_kevac(psk0, 0, scalar_eng=True)
            emit_q_mm(0)
            emit_qbuild(0, scalar_eng=True)

            # ---- phase E: attention, software-pipelined across heads,
            # with et1..3 Q/K+build embedded in the PE/VE slack ----
            oT = [
                sb.tile([128, T], bf16, tag=f"oT{m}", name=f"oT{m}") for m in range(4)
            ]
            p_tiles = {}
            psav_t = {}

            def emit_logit(h, jt):
                psl = ps.tile([128, T], f32, tag="pA", bufs=2, name=f"psl{h}_{jt}")
                for ic in range(2):
                    nc.tensor.matmul(
                        psl[:, ic * 512 : (ic + 1) * 512],
                        kbig[h][:, jt * 128 : (jt + 1) * 128],
                        qbig[h][:, ic * 512 : (ic + 1) * 512],
                        start=True,
                        stop=True,
                    )
                pexp = sb.tile([128, T], bf16, tag="P", bufs=12)
                p_tiles[(h, jt)] = pexp
                nc.scalar.activation(pexp[:], psl[:], Act.Exp, scale=0.125)

            def emit_av(h, jt):
                if jt == 0:
                    psav_t[h] = ps.tile([128, T], f32, tag="pB", bufs=1, name=f"psav{h}")
                psav = psav_t[h]
                for ic in range(2):
                    nc.tensor.matmul(
                        psav[:, ic * 512 : (ic + 1) * 512],
                        v_ext[jt][:, h * 128 : (h + 1) * 128],
                        p_tiles[(h, jt)][:, ic * 512 : (ic + 1) * 512],
                        start=(jt == 0),
                        stop=(jt == NT - 1),
                    )

            def emit_div(h):
                psav = psav_t[h]
                m = h // 2
                rr = sb.tile([64, T], f32, tag="rr", bufs=2)
                nc.vector.reciprocal_approx_fast(rr[:], psav[0:64, :])
                row0 = (h % 2) * 64
                nc.vector.tensor_tensor(
                    oT[m][row0 : row0 + 64, :], psav[64:128, :], rr[:], Alu.mult
                )

            embed = {0: 1, 2: 2, 4: 3}  # head h -> et whose build rides on it
            for h in range(H):
                for jt in range(4):
                    emit_logit(h, jt)
                    if h > 0:
                        emit_av(h - 1, 4 + jt)
                if h > 0:
                    emit_div(h - 1)
                if h in embed:
                    psk = emit_k_mm(embed[h])
                    emit_kevac(psk, embed[h], scalar_eng=False)
                for jt in range(4, 8):
                    emit_logit(h, jt)
                if h in embed:
                    emit_q_mm(embed[h])
                for jt in range(4):
                    emit_av(h, jt)
                if h in embed:
                    emit_qbuild(embed[h], scalar_eng=False)
            for jt in range(4, 8):
                emit_av(H - 1, jt)
            emit_div(H - 1)

            # ---- phase F: output projection ----
            for it in range(NT):
                psy = ps.tile([128, T], f32, tag="pA", bufs=2, name=f"psy{it}")
                for ft in range(NC):
                    nc.tensor.matmul(
                        psy[:, 0:512],
                        oT[ft][:, it * 128 : (it + 1) * 128],
                        wo[ft][:],
                        start=(ft == 0),
                        stop=(ft == NC - 1),
                    )
                yt = sb.tile([128, E], f32, tag="y", bufs=3)
                nc.vector.tensor_add(yt[:], psy[:, 0:512], ob[:])
                nc.sync.dma_start(out_t[it], yt[:])

    if not nc.is_finalized():
        nc.finalize()
    return nc


def _host_prep(inputs):
    """Fold LN gamma/beta + biases into weights; build tables. Returns in_maps."""
    import ml_dtypes

    bf = ml_dtypes.bfloat16
    x = np.asarray(inputs["input_tensor"], np.float32)  # (B, T, E)
    mask = np.asarray(inputs["sequence_mask"])  # (B, T) bool
    gamma = np.asarray(inputs["ln_scale"], np.float32)
    beta = np.asarray(inputs["ln_bias"], np.float32)
    q_w = np.asarray(inputs["q_w"], np.float32)
    k_w = np.asarray(inputs["k_w"], np.float32)
    v_w = np.asarray(inputs["v_w"], np.float32)
    in_b = np.asarray(inputs["in_proj_bias"], np.float32)
    out_w = np.asarray(inputs["out_w"], np.float32)
    out_b = np.asarray(inputs["out_b"], np.float32)
    pos_u = np.asarray(inputs["pos_bias_u"], np.float32).reshape(-1)
    pos_v = np.asarray(inputs["pos_bias_v"], np.float32).reshape(-1)

    bias_k, bias_q, bias_v = np.split(in_b, 3)  # torch unpack order

    qw_eff = q_w * gamma[None, :]
    kw_eff = k_w * gamma[None, :]
    vw_eff = v_w * gamma[None, :]
    bias_qu = q_w @ beta + bias_q + pos_u
    bias_qv = q_w @ beta + bias_q + pos_v
    bias_v_full = v_w @ beta + bias_v
    out_b_eff = out_b + out_w @ bias_v_full

    qwT = np.ascontiguousarray(qw_eff.T).astype(bf)
    kwT = np.ascontiguousarray(kw_eff.T).astype(bf)
    vwT = np.ascontiguousarray(vw_eff.T).astype(bf)
    owT = np.ascontiguousarray(out_w.T).astype(bf)

    pbias = np.zeros((128, 8), np.float32)
    for et in range(NC):
        pbias[:, et] = bias_qu[et * 128 : (et + 1) * 128]
        pbias[:, 4 + et] = bias_qv[et * 128 : (et + 1) * 128]

    inv_freq = (1.0 / (10000.0 ** (np.arange(0, DH, 2, dtype=np.float64) / DH))).astype(
        np.float64
    )
    ang = np.outer(inv_freq, np.arange(T, dtype=np.float64))  # (32, T)
    cosT = np.cos(ang)
    sinT = np.sin(ang)
    tblk = np.concatenate([cosT, sinT], axis=0).astype(bf)  # (64, T)
    tblsin = np.tile(sinT, (4, 1)).astype(bf)  # (128, T)
    tblcos = np.tile(cosT, (4, 1)).astype(bf)  # (128, T)

    obias = np.tile(out_b_eff.reshape(1, E), (128, 1)).astype(np.float32)
    ident = np.eye(128, dtype=np.float32)

    shared = {
        "qwT": qwT,
        "kwT": kwT,
        "vwT": vwT,
        "owT": owT,
        "pbias": pbias,
        "tblk": tblk,
        "tblsin": tblsin,
        "tblcos": tblcos,
        "obias": obias,
        "ident": ident,
    }
    in_maps = []
    for b in range(N_CORES):
        mt = np.zeros((128, NT), np.float32)
        mb = mask[b].astype(np.float32)
        for jt in range(NT):
            mt[:, jt] = mb[jt * 128 : (jt + 1) * 128]
        vones = np.repeat(mt.T.reshape(NT, 128, 1), 512, axis=2)  # (NT,128,512)
        vones = np.ascontiguousarray(
            vones.transpose(1, 0, 2).reshape(128, NT * 512)
        ).astype(bf)
        in_maps.append(
            {"x": np.ascontiguousarray(x[b]), "maskt": mt, "vones": vones, **shared}
        )
    return in_maps


def kernel(**inputs) -> np.ndarray:
    from concourse.bass_utils import run_bass_kernel_spmd

    in_maps = _host_prep(inputs)
    if "nc" not in _CACHE:
        _CACHE["nc"] = _build_nc()
    trace = os.environ.get("KERNEL_TRACE", "0") == "1"
    res = run_bass_kernel_spmd(
        _CACHE["nc"], in_maps, core_ids=list(range(N_CORES)), trace=trace
    )
    _CACHE["last_result"] = res
    out = np.stack([res.results[i]["out"] for i in range(N_CORES)], axis=0)
    return out.astype(np.float32)
